# revision 14
# baseline (speedup 1.0000x reference)
"""CSConv2D on 8 TRN2 NeuronCores — per-pixel 5x5 kernel selection from a
25-entry bank, applied depthwise over channels, 'same' zero padding.

Sharding: data-parallel over batch B=8, one batch element per core;
kernel_bank-derived weights are baked per-core on the host (forward pass
only — no collectives needed). Full inputs in, full output out.

Formulation: the dynamic depthwise conv becomes banded-stationary matmuls on
the TensorEngine with fp32 PSUM accumulation (operands bf16; rel err ~4e-3).

The kernel is DMA-bound (~224 GB/s effective per core), so v2 minimizes HBM
bytes vs v1:
  - x ships as slabA only [52, HP*C] per tile (7.8MB); slabB (the one-row-down
    shifted copy, partitions 52-103) is built on-device with DVE/ACT copies.
  - slot-2 stationaries (tap-row 4) have a zero upper half; only the used
    [52, TW] half is shipped (bands_c), separate from slots 0/1 (bands_ab).
  - output junk partitions 48-63 are never stored: two [48, *] DMAs per block.
  - no PSUM memset: partitions 48-63 of each bank carry garbage that is never
    read by anything that ships.

Geometry per core (1 batch element):
  - 4 column tiles of 48 output pixels; input window 52 cols each.
  - x2[t] SBUF tile [104, 196*96]: partitions 0-51 = padded w-slab rows as-is
    (slabA), partitions 52-103 = same slab shifted down one row (slabB), so a
    single AP [104, 96] at row-offset p reads input rows p and p+1 stacked.
  - Stationary band S2 [104, 48] packs tap-rows (i, i+1): rows 0-51 = banded
    weights of tap-row i, rows 52-103 = tap-row i+1. Slots per (h, t):
    (0,1), (2,3) from bands_ab; (4,) is a [52, 48] stationary from bands_c.
  - out[h, t] = sum_slot S2_slot^T @ x2[t][:, h + 2*slot, :]  (PSUM f32 accum)
"""

import numpy as np
import ml_dtypes

import concourse.bass as bass
import concourse.bacc as bacc
import concourse.mybir as mybir
from concourse.tile import TileContext
from concourse.bass_utils import run_bass_kernel_spmd

B, C, H, W = 8, 96, 192, 192
K, PAD = 5, 2
TW = 48
NT = W // TW          # 4 column tiles
WIN = TW + 2 * PAD    # 52 input cols per tile
K2 = 2 * WIN          # 104 packed contraction
HP = H + 2 * PAD      # 196 padded rows
NS = 3                # stationary slots per (h, t): tap-rows (0,1), (2,3), (4,)
RB = 8                # rows per block
HB = H // RB          # 24 blocks
BF16 = ml_dtypes.bfloat16
N_CORES = 8

_BUILD_CACHE = {}


def build_body(nc, tc, x, bands_ab, wdat_c, widx_c, out):
    with (
        tc.tile_pool(name="xpool", bufs=NT) as xpool,
        tc.tile_pool(name="bpool", bufs=3) as bpool,
        tc.tile_pool(name="cpool", bufs=3) as cpool,
        tc.tile_pool(name="wpool", bufs=3) as wpool,
        tc.tile_pool(name="ipool", bufs=1) as ipool,
        tc.tile_pool(name="opool", bufs=3) as opool,
        tc.tile_pool(name="pspool", bufs=8, space="PSUM") as pspool,
    ):
        # Constant scatter-index pattern for slot-2 stationaries, loaded once.
        ix = ipool.tile([112, 160], mybir.dt.int16)
        nc.sync.dma_start(out=ix, in_=widx_c[0:112, 0:160])
        xs = []
        for t in range(NT):
            xt = xpool.tile([K2, HP * C], mybir.dt.bfloat16, tag="xslab")
            xs.append(xt)
        # Split each slab load into row-slices, issued slice-major across
        # slabs, so every slab's first rows land before any slab's tail and
        # the first h-blocks' matmuls start early. (SBUF->SBUF slabB builds
        # were tried and serialize the DMA stream on HW — ship both halves.)
        qr = HP // 4
        for q in range(4):
            lo = q * qr
            hi = HP if q == 3 else (q + 1) * qr
            for t in range(NT):
                nc.sync.dma_start(out=xs[t][:, lo * C : hi * C],
                                  in_=x[t][:, lo * C : hi * C])
        for hb in range(HB):
            bt = bpool.tile([K2, RB * NT * 2 * TW], mybir.dt.bfloat16)
            # Alternate band loads between the SWDGE (gpsimd) and HWDGE
            # (scalar) queues — a single queue rate-limits the pipeline.
            beng = nc.gpsimd if hb % 2 == 0 else nc.scalar
            beng.dma_start(out=bt, in_=bands_ab[hb])
            # Slot-2 stationaries are GENERATED on-device: ship only the
            # ~36KB of per-pixel tap-4 weights per block and local_scatter
            # them into the zeroed banded layout on the gpsimd engine
            # (saves ~7.7MB/iter of HBM traffic).
            ct = cpool.tile([112, RB * NT * TW], mybir.dt.bfloat16)
            wtc = wpool.tile([112, 160], mybir.dt.bfloat16)
            nc.sync.dma_start(out=wtc, in_=wdat_c[hb])
            nc.gpsimd.local_scatter(
                ct[0:112, :], wtc[0:112, :], ix[0:112, :],
                channels=112, num_elems=RB * NT * TW, num_idxs=160)
            st = opool.tile([112, (RB // 2) * NT * C], mybir.dt.bfloat16)
            for r2 in range(RB // 2):
                # One PSUM bank holds an h-pair: even row at partitions 0-47,
                # odd row at 64-111 (matmul col base must be 64-aligned).
                ps = pspool.tile([112, NT * C], mybir.dt.float32)
                for par in range(2):
                    h = hb * RB + r2 * 2 + par
                    pb = par * 64
                    r = r2 * 2 + par
                    for t in range(NT):
                        for s in range(2):
                            fo = (((r * NT) + t) * 2 + s) * TW
                            nc.tensor.matmul(
                                ps[pb : pb + TW, t * C : (t + 1) * C],
                                lhsT=bt[:, fo : fo + TW],
                                rhs=xs[t][:, (h + 2 * s) * C : (h + 2 * s + 1) * C],
                                start=(s == 0),
                                stop=False,
                                skip_group_check=True,
                            )
                        fo = ((r * NT) + t) * TW
                        nc.tensor.matmul(
                            ps[pb : pb + TW, t * C : (t + 1) * C],
                            lhsT=ct[0:K2, fo : fo + TW],
                            rhs=xs[t][:, (h + 4) * C : (h + 5) * C],
                            start=False,
                            stop=True,
                            skip_group_check=True,
                        )
                dst = st[:, r2 * NT * C : (r2 + 1) * NT * C]
                if r2 % 2 == 0:
                    nc.vector.tensor_copy(dst, ps)
                else:
                    nc.scalar.copy(dst, ps)
            # Store only the real rows: even h at st[0:48], odd at st[64:112].
            oeng = nc.scalar if hb % 2 == 0 else nc.gpsimd
            oeng.dma_start(out=out[hb, 0:TW], in_=st[0:TW, :])
            oeng2 = nc.gpsimd if hb % 2 == 0 else nc.scalar
            oeng2.dma_start(out=out[hb, TW : 2 * TW], in_=st[64 : 64 + TW, :])


def build_bass():
    if "nc" in _BUILD_CACHE:
        return _BUILD_CACHE["nc"]
    nc = bacc.Bacc()
    x = nc.declare_dram_parameter("x", [NT, K2, HP * C], mybir.dt.bfloat16,
                                  isOutput=False)
    bands_ab = nc.declare_dram_parameter(
        "bands_ab", [HB, K2, RB * NT * 2 * TW], mybir.dt.bfloat16,
        isOutput=False)
    wdat_c = nc.declare_dram_parameter(
        "wdat_c", [HB, 112, 160], mybir.dt.bfloat16, isOutput=False)
    widx_c = nc.declare_dram_parameter(
        "widx_c", [112, 160], mybir.dt.int16, isOutput=False)
    out = nc.declare_dram_parameter(
        "out", [HB, 2 * TW, (RB // 2) * NT * C], mybir.dt.bfloat16,
        isOutput=True)
    with TileContext(nc) as tc:
        build_body(nc, tc, x, bands_ab, wdat_c, widx_c, out)
    nc.finalize()
    _BUILD_CACHE["nc"] = nc
    return nc


def prep_inputs(input, kernel_bank, buckets):
    input = np.asarray(input, dtype=np.float32)
    kernel_bank = np.asarray(kernel_bank, dtype=np.float32)
    buckets = np.asarray(buckets).astype(np.int64)

    # x2: padded transpose with one extra row so slabB = slabA shifted by +1.
    xt = input.transpose(0, 2, 3, 1)  # [B, H, W, C]
    xpad = np.zeros((B, HP + 1, W + 2 * PAD, C), np.float32)
    xpad[:, PAD : PAD + H, PAD : PAD + W, :] = xt
    xw = xpad.transpose(0, 2, 1, 3)  # [B, Wp, HP+1, C]
    cols = []
    for t in range(NT):
        slabA = xw[:, t * TW : t * TW + WIN, 0:HP]       # [B, 52, 196, C]
        slabB = xw[:, t * TW : t * TW + WIN, 1 : HP + 1]  # shifted by one row
        cols.append(np.concatenate([slabA, slabB], axis=1))  # [B, 104, 196, C]
    x2 = np.stack(cols, axis=1)  # [B, NT, 104, 196, C]
    x2_bf = np.ascontiguousarray(x2.reshape(B, NT, K2, HP * C)).astype(BF16)

    # Bands: per-pixel gather + banded packing. Slots 0/1 (tap-row pairs
    # (0,1), (2,3)) are [104, TW] stationaries shipped dense; slot 2
    # (tap-row 4) ships compact (5 values per partition per stationary) and
    # is expanded on-device by gpsimd local_scatter with a constant index
    # pattern.
    kbg = kernel_bank[buckets]  # [B, H, W, 5, 5]
    kbg3 = kbg.reshape(B, HB, RB, NT, TW, K, K)  # [b, hb, r, t, m, i, j]
    bnd = np.zeros((B, HB, K2, RB, NT, 2, TW), np.float32)
    marr = np.arange(TW)
    for i in range(K - 1):
        half, slot = i % 2, i // 2
        for j in range(K):
            src = kbg3[:, :, :, :, marr, i, j]  # [B, HB, RB, NT, TW]
            bnd[:, :, half * WIN + marr + j, :, :, slot, marr] = (
                src.transpose(4, 0, 1, 2, 3)
            )
    bands_ab = bnd.reshape(B, HB, K2, RB * NT * 2 * TW).astype(BF16)

    # Slot-2 compact scatter data + constant index pattern.
    # ct stationary g2 = r*NT + t at elems [g2*TW, (g2+1)*TW); partition
    # p < 52 carries w[tap4, j] for output m = p - j.
    pp = np.arange(112)[:, None, None]
    g2 = np.arange(RB * NT)[None, :, None]
    jj = np.arange(K)[None, None, :]
    rr = g2 // NT
    tt = g2 % NT
    mm = pp - jj
    valid = (pp < WIN) & (mm >= 0) & (mm < TW)
    widx = np.where(valid, g2 * TW + mm, -1).astype(np.int16)  # [112, 32, 5]
    widx_c = np.ascontiguousarray(widx.reshape(112, 160))
    mc = np.clip(mm, 0, TW - 1)
    rb_, tb_, mb_, jb_ = np.broadcast_arrays(rr, tt, mc, jj)
    vals = kbg3[:, :, rb_, tb_, mb_, 4, jb_]  # [B, HB, 112, 32, 5]
    vals = vals * valid[None, None]
    wdat_c = vals.reshape(B, HB, 112, 160).astype(BF16)

    return [
        {"x": x2_bf[b], "bands_ab": bands_ab[b], "wdat_c": wdat_c[b],
         "widx_c": widx_c}
        for b in range(B)
    ]


def unpack_output(outs):
    """outs: B x [HB, 2*TW, (RB//2)*NT*C] -> [B, C, H, W] float32."""
    o = np.stack([np.asarray(a, dtype=np.float32) for a in outs]).reshape(
        B, HB, 2, TW, RB // 2, NT, C
    )
    # o[b, hb, par, wp, r2, t, c] -> out[b, c, hb*RB + r2*2 + par, t*TW + wp]
    out = o.transpose(0, 6, 1, 4, 2, 5, 3).reshape(B, C, H, W)
    return np.ascontiguousarray(out).astype(np.float32)


def run_spmd(in_maps, trace=False, **kwargs):
    nc = build_bass()
    return run_bass_kernel_spmd(nc, in_maps, core_ids=list(range(N_CORES)),
                                trace=trace, **kwargs)


def kernel(input, kernel_bank, buckets):
    in_maps = prep_inputs(input, kernel_bank, buckets)
    res = run_spmd(in_maps)
    return unpack_output([res.results[i]["out"] for i in range(N_CORES)])
